# revision 13
# baseline (speedup 1.0000x reference)
"""Trainium2 Bass kernel for nn_MoEPredictor (moe_routing).

Data-parallel across 8 NeuronCores: tokens (B*M = 98304) are sharded into 8
contiguous chunks of 12288; the tiny router/expert weights are replicated.

On-chip layout is feature-major: activations live as [feature_part, token_free]
so every layer is matmul(psum, lhsT=W_chunk, rhs=act) with tokens as the
moving operand (512 per tile).  Top-2 gating is computed token-major after a
PE transpose of the logits; the gating weights are folded into the expert
hidden activations right before the last matmul of each head so the expert
combine falls out of PSUM accumulation for free.

Precision split: the router runs in strict fp32 (so the top-2 selection
matches the fp32 reference almost everywhere — selection flips near logit
ties dominate the output error otherwise), while the expert heads run in
bf16 (1 cycle/row on the PE vs 4 for fp32).  The gating softmax uses tanh
identities (sigmoid(z) = (1+tanh(z/2))/2, exp(z) = s/(1-s)) so every
ScalarE op stays in the gelu/tanh activation-table set - no table reloads.
"""

import os

import numpy as np

import concourse.bacc as bacc
import concourse.bass as bass  # noqa: F401
import concourse.mybir as mybir
import concourse.tile as tile
from concourse.bass import ts
from concourse.bass_utils import run_bass_kernel_spmd
from concourse.masks import make_identity

AF = mybir.ActivationFunctionType
ALU = mybir.AluOpType
AX = mybir.AxisListType
F32 = mybir.dt.float32
BF16 = mybir.dt.bfloat16

# Problem constants (hardcoded; kernel.py must be self-contained).
B, M, D, E, T = 16384, 6, 128, 6, 60
OUT_T = T * 2          # 120
R1, R2 = 256, 128      # router hidden dims
HT = 256               # traj head hidden
HS1, HS2 = 128, 64     # score head hidden
N_CORES = 8
P = 128
TILE = 512
G = TILE // P          # 4 token sub-tiles of 128
NTOK = B * M
NTOK_CORE = NTOK // N_CORES


def mm(nc, out, lhsT, rhs, start=True, stop=True):
    nc.tensor.matmul(out, lhsT, rhs, start=start, stop=stop,
                     skip_group_check=True)


def build_kernel(ntok: int) -> bacc.Bacc:
    assert ntok % TILE == 0
    n_tiles = ntok // TILE
    nc = bacc.Bacc("TRN2", target_bir_lowering=False, debug=False)

    def di(name, shape):
        return nc.dram_tensor(name, shape, F32, kind="ExternalInput").ap()

    def do(name, shape):
        return nc.dram_tensor(name, shape, F32, kind="ExternalOutput").ap()

    x = di("x", [ntok, D])
    rW1, rb1 = di("rW1", [D, R1]), di("rb1", [R1])
    rW2, rb2 = di("rW2", [R1, R2]), di("rb2", [R2])
    rW3, rb3 = di("rW3", [R2, E]), di("rb3", [E])
    tW1, tb1 = di("tW1", [E, D, HT]), di("tb1", [E, HT])
    tW2, tb2 = di("tW2", [E, HT, HT]), di("tb2", [E, HT])
    tW3, tb3 = di("tW3", [E, HT, OUT_T]), di("tb3", [E, OUT_T])
    sW1, sb1 = di("sW1", [E, D, HS1]), di("sb1", [E, HS1])
    sW2, sb2 = di("sW2", [E, HS1, HS2]), di("sb2", [E, HS2])
    sW3, sb3 = di("sW3", [E, HS2, 1]), di("sb3", [E, 1])

    out_traj = do("out_traj", [ntok, OUT_T])
    out_score = do("out_score", [ntok])
    out_probs = do("out_probs", [ntok, E])

    with tile.TileContext(nc) as tc:
        with (
            tc.tile_pool(name="consts", bufs=1) as cp,
            tc.tile_pool(name="work", bufs=3) as wp,
            tc.tile_pool(name="psum", bufs=4, space="PSUM") as pp,
            tc.tile_pool(name="dram", bufs=3, space="DRAM") as dp,
        ):
            # ---- resident constants / weights ----
            ident = cp.tile([P, P], F32, name="ident", tag="ident")
            make_identity(nc, ident)

            with nc.allow_non_contiguous_dma(reason="one-time small weight loads"):
                # router weights: strict fp32
                rW1_s = cp.tile([D, R1], F32, name="rW1_s", tag="rW1_s")
                nc.sync.dma_start(rW1_s, rW1)
                rW2_s = cp.tile([P, 2, R2], F32, name="rW2_s", tag="rW2_s")
                nc.sync.dma_start(rW2_s, rW2.rearrange("(c k) m -> k c m", k=P))
                rW3_s = cp.tile([P, E], F32, name="rW3_s", tag="rW3_s")
                nc.sync.dma_start(rW3_s, rW3)
                # expert weights: bf16 (gpsimd DMA casts f32 -> bf16)
                tW1_s = cp.tile([P, E, HT], BF16, name="tW1_s", tag="tW1_s")
                nc.gpsimd.dma_start(tW1_s, tW1.rearrange("e k m -> k e m"))
                tW2_s = cp.tile([P, E, 2, HT], BF16, name="tW2_s", tag="tW2_s")
                nc.gpsimd.dma_start(tW2_s, tW2.rearrange("e (c k) m -> k e c m", k=P))
                tW3_s = cp.tile([P, E, 2, OUT_T], BF16, name="tW3_s", tag="tW3_s")
                nc.gpsimd.dma_start(tW3_s, tW3.rearrange("e (c k) m -> k e c m", k=P))
                sW1_s = cp.tile([P, E, HS1], BF16, name="sW1_s", tag="sW1_s")
                nc.gpsimd.dma_start(sW1_s, sW1.rearrange("e k m -> k e m"))
                sW2_s = cp.tile([P, E, HS2], BF16, name="sW2_s", tag="sW2_s")
                nc.gpsimd.dma_start(sW2_s, sW2.rearrange("e k m -> k e m"))
                sW3_s = cp.tile([HS2, E], BF16, name="sW3_s", tag="sW3_s")
                nc.gpsimd.dma_start(sW3_s, sW3.rearrange("e k o -> k (e o)"))
                tb3_s = cp.tile([E, OUT_T], BF16, name="tb3_s", tag="tb3_s")
                nc.gpsimd.dma_start(tb3_s, tb3)
                sb3_s = cp.tile([E, 1], BF16, name="sb3_s", tag="sb3_s")
                nc.gpsimd.dma_start(sb3_s, sb3)

                rb1_s = cp.tile([P, 2], F32, name="rb1_s", tag="rb1_s")
                nc.sync.dma_start(rb1_s, rb1.rearrange("(c p) -> p c", p=P))
                rb2_s = cp.tile([P, 1], F32, name="rb2_s", tag="rb2_s")
                nc.sync.dma_start(rb2_s, rb2[:, None])
                rb3_s = cp.tile([E, 1], F32, name="rb3_s", tag="rb3_s")
                nc.sync.dma_start(rb3_s, rb3[:, None])
                tb1_s = cp.tile([P, E, 2], F32, name="tb1_s", tag="tb1_s")
                nc.sync.dma_start(tb1_s, tb1.rearrange("e (c p) -> p e c", p=P))
                tb2_s = cp.tile([P, E, 2], F32, name="tb2_s", tag="tb2_s")
                nc.sync.dma_start(tb2_s, tb2.rearrange("e (c p) -> p e c", p=P))
                sb1_s = cp.tile([P, E], F32, name="sb1_s", tag="sb1_s")
                nc.sync.dma_start(sb1_s, sb1.rearrange("e p -> p e"))
                sb2_s = cp.tile([HS2, E], F32, name="sb2_s", tag="sb2_s")
                nc.sync.dma_start(sb2_s, sb2.rearrange("e p -> p e"))
                # bias rows as K=1 lhsT operands for bias-matmuls (bf16),
                # plus a ones row as the moving operand
                tb1_r = cp.tile([1, E, 2, P], BF16, name="tb1_r", tag="tb1_r")
                nc.gpsimd.dma_start(tb1_r, tb1[None, :, :].rearrange(
                    "o e (c p) -> o e c p", p=P))
                tb2_r = cp.tile([1, E, 2, P], BF16, name="tb2_r", tag="tb2_r")
                nc.gpsimd.dma_start(tb2_r, tb2[None, :, :].rearrange(
                    "o e (c p) -> o e c p", p=P))
            ones_bf = cp.tile([1, TILE], BF16, name="ones_bf", tag="ones_bf")
            nc.vector.memset(ones_bf, 1.0)

            # ---- per 512-token tile ----
            for t in range(n_tiles):
                # load [512, 128] chunk token-major, PE-transpose to
                # feature-major xT [128 feat, 512 tok]
                xa = wp.tile([P, G, D], F32, name="xa", tag="xa")
                nc.sync.dma_start(
                    xa, x[ts(t, TILE), :].rearrange("(g p) d -> p g d", p=P))
                xT = wp.tile([P, TILE], F32, name="xT", tag="xT")
                for g in range(G):
                    xps = pp.tile([P, P], F32, name="xps", tag="sg", bufs=3)
                    nc.tensor.transpose(xps, xa[:, g, :], ident)
                    nc.vector.tensor_copy(xT[:, ts(g, P)], xps)
                xTb = wp.tile([P, TILE], BF16, name="xTb", tag="xTb")
                nc.vector.tensor_copy(xTb, xT)

                # ---- router MLP (fp32) ----
                r1a = pp.tile([P, TILE], F32, name="r1a", tag="sg", bufs=3)
                r1b = pp.tile([P, TILE], F32, name="r1b", tag="sg", bufs=3)
                mm(nc, r1a, rW1_s[:, 0:P], xT)
                mm(nc, r1b, rW1_s[:, P:R1], xT)
                h1 = wp.tile([P, 2, TILE], F32, name="h1", tag="h1")
                nc.scalar.activation(h1[:, 0, :], r1a, AF.Gelu, bias=rb1_s[:, 0:1])
                nc.scalar.activation(h1[:, 1, :], r1b, AF.Gelu, bias=rb1_s[:, 1:2])
                r2p = pp.tile([P, TILE], F32, name="r2p", tag="sg", bufs=3)
                mm(nc, r2p, rW2_s[:, 0, :], h1[:, 0, :], start=True, stop=False)
                mm(nc, r2p, rW2_s[:, 1, :], h1[:, 1, :], start=False, stop=True)
                h2 = wp.tile([P, TILE], F32, name="h2", tag="h2")
                nc.scalar.activation(h2, r2p, AF.Gelu, bias=rb2_s[:, 0:1])
                r3p = pp.tile([E, TILE], F32, name="r3p", tag="sg", bufs=3)
                mm(nc, r3p, rW3_s, h2)
                logit_fm = wp.tile([E, TILE], F32, name="logit_fm", tag="logit_fm")
                nc.vector.tensor_scalar(logit_fm, r3p, rb3_s, None, op0=ALU.add)

                # ---- logits to token-major, gating math ----
                ltp = pp.tile([P, G, E], F32, name="ltp", tag="sg", bufs=3)
                for g in range(G):
                    nc.tensor.transpose(ltp[:, g, :], logit_fm[:, ts(g, P)],
                                        ident[:E, :E])
                lt = wp.tile([P, G, E], F32, name="lt", tag="lt")
                nc.vector.tensor_copy(lt, ltp)

                m1 = wp.tile([P, G], F32, name="m1", tag="m1")
                nc.vector.reduce_max(m1, lt, axis=AX.X)
                m1b = m1[:, :, None].to_broadcast((P, G, E))
                top1 = wp.tile([P, G, E], F32, name="top1", tag="top1")
                nc.vector.tensor_tensor(top1, lt, m1b, op=ALU.is_ge)
                masked = wp.tile([P, G, E], F32, name="masked", tag="masked")
                nc.vector.tensor_scalar(masked, top1, -1e30, None, op0=ALU.mult)
                nc.vector.tensor_add(masked, masked, lt)
                m2 = wp.tile([P, G], F32, name="m2", tag="m2")
                nc.vector.reduce_max(m2, masked, axis=AX.X)
                m2b = m2[:, :, None].to_broadcast((P, G, E))

                # e1 = exp(lt - m1) via tanh (stays in the gelu table set):
                # s = sigmoid(z) = 0.5 + 0.5*tanh(z/2);  exp(z) = s / (1 - s)
                dd = wp.tile([P, G, E], F32, name="dd", tag="dd")
                nc.vector.tensor_sub(dd, lt, m1b)
                th = wp.tile([P, G, E], F32, name="th", tag="th")
                nc.scalar.activation(th, dd, AF.Tanh, scale=0.5)
                sg = wp.tile([P, G, E], F32, name="sg", tag="sg")
                nc.vector.tensor_scalar(sg, th, 0.5, 0.5, op0=ALU.mult, op1=ALU.add)
                omsg = wp.tile([P, G, E], F32, name="omsg", tag="omsg")
                nc.vector.tensor_scalar(omsg, th, -0.5, 0.5, op0=ALU.mult,
                                        op1=ALU.add)
                rn = wp.tile([P, G, E], F32, name="rn", tag="rn")
                nc.vector.reciprocal(rn, omsg)
                e1 = wp.tile([P, G, E], F32, name="e1", tag="e1")
                nc.vector.tensor_mul(e1, sg, rn)

                S = wp.tile([P, G], F32, name="S", tag="S")
                nc.vector.reduce_sum(S, e1, axis=AX.X)
                rS = wp.tile([P, G], F32, name="rS", tag="rS")
                nc.vector.reciprocal(rS, S)
                probs_t = wp.tile([P, G, E], F32, name="probs_t", tag="probs_t")
                nc.vector.tensor_mul(probs_t, e1,
                                     rS[:, :, None].to_broadcast((P, G, E)))
                with nc.allow_non_contiguous_dma(reason="24B prob rows"):
                    nc.sync.dma_start(
                        out_probs[ts(t, TILE), :].rearrange("(g p) e -> p g e", p=P),
                        probs_t)

                # top-2 renormalized weights:
                #   w[e] = exp(l[e]-m1) * (l[e] >= m2) / (1 + exp(m2-m1))
                # and 1/(1+exp(dm)) = sigmoid(-dm) = 0.5 - 0.5*tanh(dm/2)
                ge2 = wp.tile([P, G, E], F32, name="ge2", tag="ge2")
                nc.vector.tensor_tensor(ge2, lt, m2b, op=ALU.is_ge)
                dm = wp.tile([P, G], F32, name="dm", tag="dm")
                nc.vector.tensor_sub(dm, m2, m1)
                th2 = wp.tile([P, G], F32, name="th2", tag="th2")
                nc.scalar.activation(th2, dm, AF.Tanh, scale=0.5)
                rd = wp.tile([P, G], F32, name="rd", tag="rd")
                nc.vector.tensor_scalar(rd, th2, -0.5, 0.5, op0=ALU.mult,
                                        op1=ALU.add)
                wt = wp.tile([P, G, E], F32, name="wt", tag="wt")
                nc.vector.tensor_mul(wt, e1, ge2)
                nc.vector.tensor_mul(wt, wt, rd[:, :, None].to_broadcast((P, G, E)))

                # back to feature-major [E, 512], then broadcast across
                # partitions via stride-0 DRAM-source DMA
                wfp = pp.tile([E, TILE], F32, name="wfp", tag="sg", bufs=3)
                for g in range(G):
                    nc.tensor.transpose(wfp[:, ts(g, P)], wt[:, g, :], ident)
                w_fm = wp.tile([E, TILE], BF16, name="w_fm", tag="w_fm")
                nc.vector.tensor_copy(w_fm, wfp)
                w_dram = dp.tile([E, TILE], BF16, name="w_dram", tag="w_dram")
                nc.sync.dma_start(w_dram, w_fm)
                wb = wp.tile([P, E, TILE], BF16, name="wb", tag="wb", bufs=3)
                for e in range(E):
                    nc.sync.dma_start(
                        wb[:, e, :], w_dram[e:e + 1, :].to_broadcast((P, TILE)))

                # ---- expert heads (bf16), combine via PSUM accumulation ----
                traj_ps = pp.tile([OUT_T, TILE], F32, name="traj_ps",
                                  tag="acct", bufs=1)
                score_acc = wp.tile([1, TILE], F32, name="score_acc",
                                    tag="score_acc")
                for e in range(E):
                    # trajectory head: both 128-chunks of L1 land in one
                    # 2-bank psum pair; per-chunk biases enter as K=1
                    # bias-matmuls so a single gelu covers the pair
                    t1p = pp.tile([P, 2, TILE], F32, name="t1p", tag="pd",
                                  bufs=2)
                    for c in range(2):
                        mm(nc, t1p[:, c, :], tW1_s[:, e, c * P:(c + 1) * P],
                           xTb, start=True, stop=False)
                        mm(nc, t1p[:, c, :], tb1_r[:, e, c, :], ones_bf,
                           start=False, stop=True)
                    ht1 = wp.tile([P, 2, TILE], BF16, name="ht1", tag="ht1")
                    nc.scalar.activation(ht1, t1p, AF.Gelu)
                    t2p = pp.tile([P, 2, TILE], F32, name="t2p", tag="pd",
                                  bufs=2)
                    for c in range(2):
                        mm(nc, t2p[:, 0, :], tW2_s[:, e, c, 0:P], ht1[:, c, :],
                           start=(c == 0), stop=False)
                        mm(nc, t2p[:, 1, :], tW2_s[:, e, c, P:HT], ht1[:, c, :],
                           start=(c == 0), stop=False)
                    mm(nc, t2p[:, 0, :], tb2_r[:, e, 0, :], ones_bf,
                       start=False, stop=True)
                    mm(nc, t2p[:, 1, :], tb2_r[:, e, 1, :], ones_bf,
                       start=False, stop=True)
                    ht2w = wp.tile([P, 2, TILE], BF16, name="ht2w", tag="ht2w")
                    nc.scalar.activation(ht2w, t2p, AF.Gelu)
                    nc.vector.tensor_mul(ht2w[:, 0, :], ht2w[:, 0, :], wb[:, e, :])
                    nc.vector.tensor_mul(ht2w[:, 1, :], ht2w[:, 1, :], wb[:, e, :])
                    mm(nc, traj_ps, tW3_s[:, e, 0, :], ht2w[:, 0, :],
                       start=(e == 0), stop=False)
                    mm(nc, traj_ps, tW3_s[:, e, 1, :], ht2w[:, 1, :],
                       start=False, stop=False)

                    # score head
                    s1p = pp.tile([P, TILE], F32, name="s1p", tag="sg", bufs=3)
                    mm(nc, s1p, sW1_s[:, e, :], xTb)
                    hs1 = wp.tile([P, TILE], BF16, name="hs1", tag="hs1")
                    nc.scalar.activation(hs1, s1p, AF.Gelu, bias=sb1_s[:, e:e + 1])
                    s2p = pp.tile([HS2, TILE], F32, name="s2p", tag="sg", bufs=3)
                    mm(nc, s2p, sW2_s[:, e, :], hs1)
                    hs2w = wp.tile([HS2, TILE], BF16, name="hs2w", tag="hs2w")
                    nc.scalar.activation(hs2w, s2p, AF.Gelu, bias=sb2_s[:, e:e + 1])
                    nc.vector.tensor_mul(hs2w, hs2w, wb[0:HS2, e, :])
                    sc = pp.tile([1, TILE], F32, name="sc", tag="sg", bufs=3)
                    if e < E - 1:
                        mm(nc, sc, sW3_s[:, e:e + 1], hs2w)
                    else:
                        mm(nc, sc, sW3_s[:, e:e + 1], hs2w,
                           start=True, stop=False)
                        mm(nc, sc, sb3_s, w_fm, start=False, stop=True)
                    if e == 0:
                        nc.vector.tensor_copy(score_acc, sc)
                    else:
                        nc.vector.tensor_add(score_acc, score_acc, sc)

                # bias contribution: traj += tb3.T @ w_fm
                mm(nc, traj_ps, tb3_s, w_fm, start=False, stop=True)

                # ---- outputs ----
                nc.sync.dma_start(out_score[ts(t, TILE)][None, :], score_acc)

                trajsb = wp.tile([OUT_T, TILE], F32, name="trajsb", tag="trajsb")
                nc.vector.tensor_copy(trajsb, traj_ps)
                outp = pp.tile([P, G, OUT_T], F32, name="outp", tag="acct", bufs=1)
                for g in range(G):
                    nc.tensor.transpose(outp[:, g, :], trajsb[:, ts(g, P)],
                                        ident[:OUT_T, :OUT_T])
                outsb = wp.tile([P, G, OUT_T], F32, name="outsb", tag="outsb")
                nc.vector.tensor_copy(outsb, outp)
                nc.sync.dma_start(
                    out_traj[ts(t, TILE), :].rearrange("(g p) o -> p g o", p=P),
                    outsb)
    nc.compile()
    return nc


_nc_cache: dict[int, bacc.Bacc] = {}


def get_nc(ntok: int) -> bacc.Bacc:
    if ntok not in _nc_cache:
        _nc_cache[ntok] = build_kernel(ntok)
    return _nc_cache[ntok]


last_results = None  # BassKernelResults of the most recent kernel() call


def _maybe_install_ntff_hook() -> bool:
    """Synthesize antenv.axon_hooks (absent on this image) so NTFF profiling
    works under axon.  Returns True when tracing is usable."""
    import sys
    import types

    try:
        from antenv.axon_hooks import get_axon_ntff_profile_hook  # noqa: F401
        return True
    except ImportError:
        pass
    try:
        import trn_agent_boot.trn_boot as tb

        hook = tb._ntff_profile_via_ctypes("/opt/axon/libaxon_pjrt.so")
        if hook is None:
            return False
        mod = types.ModuleType("antenv.axon_hooks")
        mod._hook = hook
        mod.get_axon_ntff_profile_hook = lambda: mod._hook
        mod.set_axon_ntff_profile_hook = lambda h: setattr(mod, "_hook", h)
        sys.modules["antenv.axon_hooks"] = mod
        import antenv

        antenv.axon_hooks = mod
        return True
    except Exception:
        return False


def kernel(**inputs) -> tuple:
    global last_results
    arrs = {k: np.ascontiguousarray(np.asarray(v, dtype=np.float32))
            for k, v in inputs.items()}
    x_full = arrs.pop("mode_features").reshape(NTOK, D)

    nc = get_nc(NTOK_CORE)
    in_maps = []
    for c in range(N_CORES):
        im = dict(arrs)
        im["x"] = x_full[c * NTOK_CORE:(c + 1) * NTOK_CORE]
        in_maps.append(im)

    want_trace = os.environ.get("BASS_KERNEL_TRACE", "0") == "1"
    trace = want_trace and _maybe_install_ntff_hook()
    if not trace:
        # keep a stray BASS_TRACE=1 in the environment from crashing the
        # axon trace path (it needs the hook module synthesized above)
        os.environ["BASS_NEVER_TRACE"] = "1"

    res = run_bass_kernel_spmd(
        nc, in_maps, core_ids=list(range(N_CORES)),
        trace=trace,
    )
    last_results = res

    traj = np.concatenate([r["out_traj"] for r in res.results], axis=0)
    score = np.concatenate([r["out_score"] for r in res.results], axis=0)
    probs = np.concatenate([r["out_probs"] for r in res.results], axis=0)

    trajectories = traj.reshape(B, M, T, 2)
    scores = score.reshape(B, M)
    probs_out = probs.reshape(B, M, E)
    avg = probs.reshape(NTOK, E).mean(axis=0)
    aux_loss = np.float32(E * np.sum(avg * avg))
    return trajectories, scores, aux_loss, probs_out


# revision 15
# speedup vs baseline: 1.3073x; 1.3073x over previous
"""Trainium2 Bass kernel for nn_MoEPredictor (moe_routing).

Data-parallel across 8 NeuronCores: tokens (B*M = 98304) are sharded into 8
contiguous chunks of 12288; the tiny router/expert weights are replicated.

On-chip layout is feature-major: activations live as [feature_part, token_free]
so every layer is matmul(psum, lhsT=W_chunk, rhs=act) with tokens as the
moving operand (512 per tile).  Top-2 gating is computed token-major after a
PE transpose of the logits; the gating weights are folded into the expert
hidden activations right before the last matmul of each head so the expert
combine falls out of PSUM accumulation for free.

Precision split: the router runs in strict fp32 (so the top-2 selection
matches the fp32 reference almost everywhere — selection flips near logit
ties dominate the output error otherwise), while the expert heads run in
bf16 (1 cycle/row on the PE vs 4 for fp32).  The gating softmax uses tanh
identities (sigmoid(z) = (1+tanh(z/2))/2, exp(z) = s/(1-s)) so every
ScalarE op stays in the gelu/tanh activation-table set - no table reloads.
"""

import os

import numpy as np

import concourse.bacc as bacc
import concourse.bass as bass  # noqa: F401
import concourse.mybir as mybir
import concourse.tile as tile
from concourse.bass import ts
from concourse.bass_utils import run_bass_kernel_spmd
from concourse.masks import make_identity

AF = mybir.ActivationFunctionType
ALU = mybir.AluOpType
AX = mybir.AxisListType
F32 = mybir.dt.float32
BF16 = mybir.dt.bfloat16

# Problem constants (hardcoded; kernel.py must be self-contained).
B, M, D, E, T = 16384, 6, 128, 6, 60
OUT_T = T * 2          # 120
R1, R2 = 256, 128      # router hidden dims
HT = 256               # traj head hidden
HS1, HS2 = 128, 64     # score head hidden
N_CORES = 8
P = 128
TILE = 512
G = TILE // P          # 4 token sub-tiles of 128
NTOK = B * M
NTOK_CORE = NTOK // N_CORES


def mm(nc, out, lhsT, rhs, start=True, stop=True):
    nc.tensor.matmul(out, lhsT, rhs, start=start, stop=stop,
                     skip_group_check=True)


def build_kernel(ntok: int) -> bacc.Bacc:
    assert ntok % TILE == 0
    n_tiles = ntok // TILE
    nc = bacc.Bacc("TRN2", target_bir_lowering=False, debug=False)

    def di(name, shape):
        return nc.dram_tensor(name, shape, F32, kind="ExternalInput").ap()

    def do(name, shape):
        return nc.dram_tensor(name, shape, F32, kind="ExternalOutput").ap()

    x = di("x", [ntok, D])
    rW1, rb1 = di("rW1", [D, R1]), di("rb1", [R1])
    rW2, rb2 = di("rW2", [R1, R2]), di("rb2", [R2])
    rW3, rb3 = di("rW3", [R2, E]), di("rb3", [E])
    tW1, tb1 = di("tW1", [E, D, HT]), di("tb1", [E, HT])
    tW2, tb2 = di("tW2", [E, HT, HT]), di("tb2", [E, HT])
    tW3, tb3 = di("tW3", [E, HT, OUT_T]), di("tb3", [E, OUT_T])
    sW1, sb1 = di("sW1", [E, D, HS1]), di("sb1", [E, HS1])
    sW2, sb2 = di("sW2", [E, HS1, HS2]), di("sb2", [E, HS2])
    sW3, sb3 = di("sW3", [E, HS2, 1]), di("sb3", [E, 1])

    out_traj = do("out_traj", [ntok, OUT_T])
    out_score = do("out_score", [ntok])
    out_probs = do("out_probs", [ntok, E])

    with tile.TileContext(nc) as tc:
        with (
            tc.tile_pool(name="consts", bufs=1) as cp,
            tc.tile_pool(name="work", bufs=4) as wp,
            tc.tile_pool(name="psum", bufs=4, space="PSUM") as pp,
            tc.tile_pool(name="dram", bufs=4, space="DRAM") as dp,
        ):
            # ---- resident constants / weights ----
            ident = cp.tile([P, P], F32, name="ident", tag="ident")
            make_identity(nc, ident)

            with nc.allow_non_contiguous_dma(reason="one-time small weight loads"):
                # router weights: strict fp32
                rW1_s = cp.tile([D, R1], F32, name="rW1_s", tag="rW1_s")
                nc.sync.dma_start(rW1_s, rW1)
                rW2_s = cp.tile([P, 2, R2], F32, name="rW2_s", tag="rW2_s")
                nc.sync.dma_start(rW2_s, rW2.rearrange("(c k) m -> k c m", k=P))
                rW3_s = cp.tile([P, E], F32, name="rW3_s", tag="rW3_s")
                nc.sync.dma_start(rW3_s, rW3)
                # expert weights: bf16 (gpsimd DMA casts f32 -> bf16)
                tW1_s = cp.tile([P, E, HT], BF16, name="tW1_s", tag="tW1_s")
                nc.gpsimd.dma_start(tW1_s, tW1.rearrange("e k m -> k e m"))
                tW2_s = cp.tile([P, E, 2, HT], BF16, name="tW2_s", tag="tW2_s")
                nc.gpsimd.dma_start(tW2_s, tW2.rearrange("e (c k) m -> k e c m", k=P))
                tW3_s = cp.tile([P, E, 2, OUT_T], BF16, name="tW3_s", tag="tW3_s")
                nc.gpsimd.dma_start(tW3_s, tW3.rearrange("e (c k) m -> k e c m", k=P))
                sW1_s = cp.tile([P, E, HS1], BF16, name="sW1_s", tag="sW1_s")
                nc.gpsimd.dma_start(sW1_s, sW1.rearrange("e k m -> k e m"))
                sW2_s = cp.tile([P, E, HS2], BF16, name="sW2_s", tag="sW2_s")
                nc.gpsimd.dma_start(sW2_s, sW2.rearrange("e k m -> k e m"))
                sW3_s = cp.tile([HS2, E], BF16, name="sW3_s", tag="sW3_s")
                nc.gpsimd.dma_start(sW3_s, sW3.rearrange("e k o -> k (e o)"))
                tb3_s = cp.tile([E, OUT_T], BF16, name="tb3_s", tag="tb3_s")
                nc.gpsimd.dma_start(tb3_s, tb3)
                sb3_s = cp.tile([E, 1], BF16, name="sb3_s", tag="sb3_s")
                nc.gpsimd.dma_start(sb3_s, sb3)

                rb1_s = cp.tile([P, 2], F32, name="rb1_s", tag="rb1_s")
                nc.sync.dma_start(rb1_s, rb1.rearrange("(c p) -> p c", p=P))
                rb2_s = cp.tile([P, 1], F32, name="rb2_s", tag="rb2_s")
                nc.sync.dma_start(rb2_s, rb2[:, None])
                rb3_s = cp.tile([E, 1], F32, name="rb3_s", tag="rb3_s")
                nc.sync.dma_start(rb3_s, rb3[:, None])
                tb1_s = cp.tile([P, E, 2], F32, name="tb1_s", tag="tb1_s")
                nc.sync.dma_start(tb1_s, tb1.rearrange("e (c p) -> p e c", p=P))
                tb2_s = cp.tile([P, E, 2], F32, name="tb2_s", tag="tb2_s")
                nc.sync.dma_start(tb2_s, tb2.rearrange("e (c p) -> p e c", p=P))
                sb1_s = cp.tile([P, E], F32, name="sb1_s", tag="sb1_s")
                nc.sync.dma_start(sb1_s, sb1.rearrange("e p -> p e"))
                sb2_s = cp.tile([HS2, E], F32, name="sb2_s", tag="sb2_s")
                nc.sync.dma_start(sb2_s, sb2.rearrange("e p -> p e"))

            # ---- per 512-token tile ----
            for t in range(n_tiles):
                # load [512, 128] chunk token-major, PE-transpose to
                # feature-major xT [128 feat, 512 tok]
                xa = wp.tile([P, G, D], F32, name="xa", tag="xa")
                nc.sync.dma_start(
                    xa, x[ts(t, TILE), :].rearrange("(g p) d -> p g d", p=P))
                xT = wp.tile([P, TILE], F32, name="xT", tag="xT")
                for g in range(G):
                    xps = pp.tile([P, P], F32, name="xps", tag="rt", bufs=3)
                    nc.tensor.transpose(xps, xa[:, g, :], ident)
                    nc.vector.tensor_copy(xT[:, ts(g, P)], xps)
                xTb = wp.tile([P, TILE], BF16, name="xTb", tag="xTb")
                nc.vector.tensor_copy(xTb, xT)

                # ---- router MLP (fp32) ----
                r1a = pp.tile([P, TILE], F32, name="r1a", tag="rt", bufs=3)
                r1b = pp.tile([P, TILE], F32, name="r1b", tag="rt", bufs=3)
                mm(nc, r1a, rW1_s[:, 0:P], xT)
                mm(nc, r1b, rW1_s[:, P:R1], xT)
                h1 = wp.tile([P, 2, TILE], F32, name="h1", tag="h1")
                nc.scalar.activation(h1[:, 0, :], r1a, AF.Gelu, bias=rb1_s[:, 0:1])
                nc.scalar.activation(h1[:, 1, :], r1b, AF.Gelu, bias=rb1_s[:, 1:2])
                r2p = pp.tile([P, TILE], F32, name="r2p", tag="rt", bufs=3)
                mm(nc, r2p, rW2_s[:, 0, :], h1[:, 0, :], start=True, stop=False)
                mm(nc, r2p, rW2_s[:, 1, :], h1[:, 1, :], start=False, stop=True)
                h2 = wp.tile([P, TILE], F32, name="h2", tag="h2")
                nc.scalar.activation(h2, r2p, AF.Gelu, bias=rb2_s[:, 0:1])
                r3p = pp.tile([E, TILE], F32, name="r3p", tag="rt", bufs=3)
                mm(nc, r3p, rW3_s, h2)
                logit_fm = wp.tile([E, TILE], F32, name="logit_fm", tag="logit_fm")
                nc.vector.tensor_scalar(logit_fm, r3p, rb3_s, None, op0=ALU.add)

                # ---- logits to token-major, gating math ----
                ltp = pp.tile([P, G, E], F32, name="ltp", tag="rt", bufs=3)
                for g in range(G):
                    nc.tensor.transpose(ltp[:, g, :], logit_fm[:, ts(g, P)],
                                        ident[:E, :E])
                lt = wp.tile([P, G, E], F32, name="lt", tag="lt")
                nc.vector.tensor_copy(lt, ltp)

                m1 = wp.tile([P, G], F32, name="m1", tag="m1")
                nc.vector.reduce_max(m1, lt, axis=AX.X)
                m1b = m1[:, :, None].to_broadcast((P, G, E))
                top1 = wp.tile([P, G, E], F32, name="top1", tag="top1")
                nc.vector.tensor_tensor(top1, lt, m1b, op=ALU.is_ge)
                masked = wp.tile([P, G, E], F32, name="masked", tag="masked")
                nc.vector.tensor_scalar(masked, top1, -1e30, None, op0=ALU.mult)
                nc.vector.tensor_add(masked, masked, lt)
                m2 = wp.tile([P, G], F32, name="m2", tag="m2")
                nc.vector.reduce_max(m2, masked, axis=AX.X)
                m2b = m2[:, :, None].to_broadcast((P, G, E))

                # e1 = exp(lt - m1) via tanh (stays in the gelu table set):
                # s = sigmoid(z) = 0.5 + 0.5*tanh(z/2);  exp(z) = s / (1 - s)
                dd = wp.tile([P, G, E], F32, name="dd", tag="dd")
                nc.vector.tensor_sub(dd, lt, m1b)
                th = wp.tile([P, G, E], F32, name="th", tag="th")
                nc.scalar.activation(th, dd, AF.Tanh, scale=0.5)
                sg = wp.tile([P, G, E], F32, name="sg", tag="sg")
                nc.vector.tensor_scalar(sg, th, 0.5, 0.5, op0=ALU.mult, op1=ALU.add)
                omsg = wp.tile([P, G, E], F32, name="omsg", tag="omsg")
                nc.vector.tensor_scalar(omsg, th, -0.5, 0.5, op0=ALU.mult,
                                        op1=ALU.add)
                rn = wp.tile([P, G, E], F32, name="rn", tag="rn")
                nc.vector.reciprocal(rn, omsg)
                e1 = wp.tile([P, G, E], F32, name="e1", tag="e1")
                nc.vector.tensor_mul(e1, sg, rn)

                S = wp.tile([P, G], F32, name="S", tag="S")
                nc.vector.reduce_sum(S, e1, axis=AX.X)
                rS = wp.tile([P, G], F32, name="rS", tag="rS")
                nc.vector.reciprocal(rS, S)
                probs_t = wp.tile([P, G, E], F32, name="probs_t", tag="probs_t")
                nc.vector.tensor_mul(probs_t, e1,
                                     rS[:, :, None].to_broadcast((P, G, E)))
                with nc.allow_non_contiguous_dma(reason="24B prob rows"):
                    nc.sync.dma_start(
                        out_probs[ts(t, TILE), :].rearrange("(g p) e -> p g e", p=P),
                        probs_t)

                # top-2 renormalized weights:
                #   w[e] = exp(l[e]-m1) * (l[e] >= m2) / (1 + exp(m2-m1))
                # and 1/(1+exp(dm)) = sigmoid(-dm) = 0.5 - 0.5*tanh(dm/2)
                ge2 = wp.tile([P, G, E], F32, name="ge2", tag="ge2")
                nc.vector.tensor_tensor(ge2, lt, m2b, op=ALU.is_ge)
                dm = wp.tile([P, G], F32, name="dm", tag="dm")
                nc.vector.tensor_sub(dm, m2, m1)
                th2 = wp.tile([P, G], F32, name="th2", tag="th2")
                nc.scalar.activation(th2, dm, AF.Tanh, scale=0.5)
                rd = wp.tile([P, G], F32, name="rd", tag="rd")
                nc.vector.tensor_scalar(rd, th2, -0.5, 0.5, op0=ALU.mult,
                                        op1=ALU.add)
                wt = wp.tile([P, G, E], F32, name="wt", tag="wt")
                nc.vector.tensor_mul(wt, e1, ge2)
                nc.vector.tensor_mul(wt, wt, rd[:, :, None].to_broadcast((P, G, E)))

                # back to feature-major [E, 512], then broadcast across
                # partitions via stride-0 DRAM-source DMA
                wfp = pp.tile([E, TILE], F32, name="wfp", tag="rt", bufs=3)
                for g in range(G):
                    nc.tensor.transpose(wfp[:, ts(g, P)], wt[:, g, :], ident)
                w_fm = wp.tile([E, TILE], BF16, name="w_fm", tag="w_fm")
                nc.vector.tensor_copy(w_fm, wfp)
                w_dram = dp.tile([E, TILE], BF16, name="w_dram", tag="w_dram")
                nc.sync.dma_start(w_dram, w_fm)
                wb = wp.tile([P, E, TILE], BF16, name="wb", tag="wb", bufs=3)
                for e in range(E):
                    nc.sync.dma_start(
                        wb[:, e, :], w_dram[e:e + 1, :].to_broadcast((P, TILE)))

                # ---- expert heads (bf16), combine via PSUM accumulation ----
                traj_ps = pp.tile([OUT_T, TILE], F32, name="traj_ps",
                                  tag="acct", bufs=1)
                score_acc = wp.tile([1, TILE], F32, name="score_acc",
                                    tag="score_acc")
                for e in range(E):
                    # trajectory head
                    t1a = pp.tile([P, TILE], F32, name="t1a", tag="ps")
                    t1b = pp.tile([P, TILE], F32, name="t1b", tag="ps")
                    mm(nc, t1a, tW1_s[:, e, 0:P], xTb)
                    mm(nc, t1b, tW1_s[:, e, P:HT], xTb)
                    ht1 = wp.tile([P, 2, TILE], BF16, name="ht1", tag="ht1")
                    nc.scalar.activation(ht1[:, 0, :], t1a, AF.Gelu,
                                         bias=tb1_s[:, e, 0:1])
                    nc.scalar.activation(ht1[:, 1, :], t1b, AF.Gelu,
                                         bias=tb1_s[:, e, 1:2])
                    t2a = pp.tile([P, TILE], F32, name="t2a", tag="ps")
                    t2b = pp.tile([P, TILE], F32, name="t2b", tag="ps")
                    for c in range(2):
                        mm(nc, t2a, tW2_s[:, e, c, 0:P], ht1[:, c, :],
                           start=(c == 0), stop=(c == 1))
                        mm(nc, t2b, tW2_s[:, e, c, P:HT], ht1[:, c, :],
                           start=(c == 0), stop=(c == 1))
                    ht2w = wp.tile([P, 2, TILE], BF16, name="ht2w", tag="ht2w")
                    nc.scalar.activation(ht2w[:, 0, :], t2a, AF.Gelu,
                                         bias=tb2_s[:, e, 0:1])
                    nc.scalar.activation(ht2w[:, 1, :], t2b, AF.Gelu,
                                         bias=tb2_s[:, e, 1:2])
                    nc.vector.tensor_mul(ht2w[:, 0, :], ht2w[:, 0, :], wb[:, e, :])
                    nc.vector.tensor_mul(ht2w[:, 1, :], ht2w[:, 1, :], wb[:, e, :])
                    mm(nc, traj_ps, tW3_s[:, e, 0, :], ht2w[:, 0, :],
                       start=(e == 0), stop=False)
                    mm(nc, traj_ps, tW3_s[:, e, 1, :], ht2w[:, 1, :],
                       start=False, stop=False)

                    # score head
                    s1p = pp.tile([P, TILE], F32, name="s1p", tag="ps")
                    mm(nc, s1p, sW1_s[:, e, :], xTb)
                    hs1 = wp.tile([P, TILE], BF16, name="hs1", tag="hs1")
                    nc.scalar.activation(hs1, s1p, AF.Gelu, bias=sb1_s[:, e:e + 1])
                    s2p = pp.tile([HS2, TILE], F32, name="s2p", tag="ps")
                    mm(nc, s2p, sW2_s[:, e, :], hs1)
                    hs2w = wp.tile([HS2, TILE], BF16, name="hs2w", tag="hs2w")
                    nc.scalar.activation(hs2w, s2p, AF.Gelu, bias=sb2_s[:, e:e + 1])
                    nc.vector.tensor_mul(hs2w, hs2w, wb[0:HS2, e, :])
                    sc = pp.tile([1, TILE], F32, name="sc", tag="rt", bufs=3)
                    if e < E - 1:
                        mm(nc, sc, sW3_s[:, e:e + 1], hs2w)
                    else:
                        mm(nc, sc, sW3_s[:, e:e + 1], hs2w,
                           start=True, stop=False)
                        mm(nc, sc, sb3_s, w_fm, start=False, stop=True)
                    if e == 0:
                        nc.vector.tensor_copy(score_acc, sc)
                    else:
                        nc.vector.tensor_add(score_acc, score_acc, sc)

                # bias contribution: traj += tb3.T @ w_fm
                mm(nc, traj_ps, tb3_s, w_fm, start=False, stop=True)

                # ---- outputs ----
                nc.sync.dma_start(out_score[ts(t, TILE)][None, :], score_acc)

                trajsb = wp.tile([OUT_T, TILE], F32, name="trajsb", tag="trajsb")
                nc.vector.tensor_copy(trajsb, traj_ps)
                outp = pp.tile([P, G, OUT_T], F32, name="outp", tag="ps")
                for g in range(G):
                    nc.tensor.transpose(outp[:, g, :], trajsb[:, ts(g, P)],
                                        ident[:OUT_T, :OUT_T])
                outsb = wp.tile([P, G, OUT_T], F32, name="outsb", tag="outsb")
                nc.vector.tensor_copy(outsb, outp)
                nc.sync.dma_start(
                    out_traj[ts(t, TILE), :].rearrange("(g p) o -> p g o", p=P),
                    outsb)
    nc.compile()
    return nc


_nc_cache: dict[int, bacc.Bacc] = {}


def get_nc(ntok: int) -> bacc.Bacc:
    if ntok not in _nc_cache:
        _nc_cache[ntok] = build_kernel(ntok)
    return _nc_cache[ntok]


last_results = None  # BassKernelResults of the most recent kernel() call


def _maybe_install_ntff_hook() -> bool:
    """Synthesize antenv.axon_hooks (absent on this image) so NTFF profiling
    works under axon.  Returns True when tracing is usable."""
    import sys
    import types

    try:
        from antenv.axon_hooks import get_axon_ntff_profile_hook  # noqa: F401
        return True
    except ImportError:
        pass
    try:
        import trn_agent_boot.trn_boot as tb

        hook = tb._ntff_profile_via_ctypes("/opt/axon/libaxon_pjrt.so")
        if hook is None:
            return False
        mod = types.ModuleType("antenv.axon_hooks")
        mod._hook = hook
        mod.get_axon_ntff_profile_hook = lambda: mod._hook
        mod.set_axon_ntff_profile_hook = lambda h: setattr(mod, "_hook", h)
        sys.modules["antenv.axon_hooks"] = mod
        import antenv

        antenv.axon_hooks = mod
        return True
    except Exception:
        return False


def kernel(**inputs) -> tuple:
    global last_results
    arrs = {k: np.ascontiguousarray(np.asarray(v, dtype=np.float32))
            for k, v in inputs.items()}
    x_full = arrs.pop("mode_features").reshape(NTOK, D)

    nc = get_nc(NTOK_CORE)
    in_maps = []
    for c in range(N_CORES):
        im = dict(arrs)
        im["x"] = x_full[c * NTOK_CORE:(c + 1) * NTOK_CORE]
        in_maps.append(im)

    want_trace = os.environ.get("BASS_KERNEL_TRACE", "0") == "1"
    trace = want_trace and _maybe_install_ntff_hook()
    if not trace:
        # keep a stray BASS_TRACE=1 in the environment from crashing the
        # axon trace path (it needs the hook module synthesized above)
        os.environ["BASS_NEVER_TRACE"] = "1"

    res = run_bass_kernel_spmd(
        nc, in_maps, core_ids=list(range(N_CORES)),
        trace=trace,
    )
    last_results = res

    traj = np.concatenate([r["out_traj"] for r in res.results], axis=0)
    score = np.concatenate([r["out_score"] for r in res.results], axis=0)
    probs = np.concatenate([r["out_probs"] for r in res.results], axis=0)

    trajectories = traj.reshape(B, M, T, 2)
    scores = score.reshape(B, M)
    probs_out = probs.reshape(B, M, E)
    avg = probs.reshape(NTOK, E).mean(axis=0)
    aux_loss = np.float32(E * np.sum(avg * avg))
    return trajectories, scores, aux_loss, probs_out


# revision 16
# speedup vs baseline: 1.6949x; 1.2965x over previous
"""Trainium2 Bass kernel for nn_MoEPredictor (moe_routing).

Data-parallel across 8 NeuronCores: tokens (B*M = 98304) are sharded into 8
contiguous chunks of 12288; the tiny router/expert weights are replicated.

On-chip layout is feature-major: activations live as [feature_part, token_free]
so every layer is matmul(psum, lhsT=W_chunk, rhs=act) with tokens as the
moving operand (512 per tile).  Top-2 gating is computed token-major after a
PE transpose of the logits; the gating weights are folded into the expert
hidden activations right before the last matmul of each head so the expert
combine falls out of PSUM accumulation for free.

Precision split: the router runs in strict fp32 (so the top-2 selection
matches the fp32 reference almost everywhere — selection flips near logit
ties dominate the output error otherwise), while the expert heads run in
bf16 (1 cycle/row on the PE vs 4 for fp32).  The gating softmax uses tanh
identities (sigmoid(z) = (1+tanh(z/2))/2, exp(z) = s/(1-s)) so every
ScalarE op stays in the gelu/tanh activation-table set - no table reloads.
"""

import os

import numpy as np

import concourse.bacc as bacc
import concourse.bass as bass  # noqa: F401
import concourse.mybir as mybir
import concourse.tile as tile
from concourse.bass import ts
from concourse.bass_utils import run_bass_kernel_spmd
from concourse.masks import make_identity

AF = mybir.ActivationFunctionType
ALU = mybir.AluOpType
AX = mybir.AxisListType
F32 = mybir.dt.float32
BF16 = mybir.dt.bfloat16

# Problem constants (hardcoded; kernel.py must be self-contained).
B, M, D, E, T = 16384, 6, 128, 6, 60
OUT_T = T * 2          # 120
R1, R2 = 256, 128      # router hidden dims
HT = 256               # traj head hidden
HS1, HS2 = 128, 64     # score head hidden
N_CORES = 8
P = 128
TILE = 512
G = TILE // P          # 4 token sub-tiles of 128
NTOK = B * M
NTOK_CORE = NTOK // N_CORES


def mm(nc, out, lhsT, rhs, start=True, stop=True):
    nc.tensor.matmul(out, lhsT, rhs, start=start, stop=stop,
                     skip_group_check=True)


def build_kernel(ntok: int) -> bacc.Bacc:
    assert ntok % TILE == 0
    n_tiles = ntok // TILE
    nc = bacc.Bacc("TRN2", target_bir_lowering=False, debug=False)

    def di(name, shape):
        return nc.dram_tensor(name, shape, F32, kind="ExternalInput").ap()

    def do(name, shape):
        return nc.dram_tensor(name, shape, F32, kind="ExternalOutput").ap()

    x = di("x", [ntok, D])
    rW1, rb1 = di("rW1", [D, R1]), di("rb1", [R1])
    rW2, rb2 = di("rW2", [R1, R2]), di("rb2", [R2])
    rW3, rb3 = di("rW3", [R2, E]), di("rb3", [E])
    tW1, tb1 = di("tW1", [E, D, HT]), di("tb1", [E, HT])
    tW2, tb2 = di("tW2", [E, HT, HT]), di("tb2", [E, HT])
    tW3, tb3 = di("tW3", [E, HT, OUT_T]), di("tb3", [E, OUT_T])
    sW1, sb1 = di("sW1", [E, D, HS1]), di("sb1", [E, HS1])
    sW2, sb2 = di("sW2", [E, HS1, HS2]), di("sb2", [E, HS2])
    sW3, sb3 = di("sW3", [E, HS2, 1]), di("sb3", [E, 1])

    out_traj = do("out_traj", [ntok, OUT_T])
    out_score = do("out_score", [ntok])
    out_probs = do("out_probs", [ntok, E])

    with tile.TileContext(nc) as tc:
        with (
            tc.tile_pool(name="consts", bufs=1) as cp,
            tc.tile_pool(name="work", bufs=3) as wp,
            tc.tile_pool(name="psum", bufs=4, space="PSUM") as pp,
            tc.tile_pool(name="dram", bufs=3, space="DRAM") as dp,
        ):
            # ---- resident constants / weights ----
            ident = cp.tile([P, P], F32, name="ident", tag="ident")
            make_identity(nc, ident)

            with nc.allow_non_contiguous_dma(reason="one-time small weight loads"):
                # router weights: strict fp32
                rW1_s = cp.tile([D, R1], F32, name="rW1_s", tag="rW1_s")
                nc.sync.dma_start(rW1_s, rW1)
                rW2_s = cp.tile([P, 2, R2], F32, name="rW2_s", tag="rW2_s")
                nc.sync.dma_start(rW2_s, rW2.rearrange("(c k) m -> k c m", k=P))
                rW3_s = cp.tile([P, E], F32, name="rW3_s", tag="rW3_s")
                nc.sync.dma_start(rW3_s, rW3)
                # expert weights: bf16 (gpsimd DMA casts f32 -> bf16)
                tW1_s = cp.tile([P, E, HT], BF16, name="tW1_s", tag="tW1_s")
                nc.gpsimd.dma_start(tW1_s, tW1.rearrange("e k m -> k e m"))
                tW2_s = cp.tile([P, E, 2, HT], BF16, name="tW2_s", tag="tW2_s")
                nc.gpsimd.dma_start(tW2_s, tW2.rearrange("e (c k) m -> k e c m", k=P))
                tW3_s = cp.tile([P, E, 2, OUT_T], BF16, name="tW3_s", tag="tW3_s")
                nc.gpsimd.dma_start(tW3_s, tW3.rearrange("e (c k) m -> k e c m", k=P))
                sW1_s = cp.tile([P, E, HS1], BF16, name="sW1_s", tag="sW1_s")
                nc.gpsimd.dma_start(sW1_s, sW1.rearrange("e k m -> k e m"))
                sW2_s = cp.tile([P, E, HS2], BF16, name="sW2_s", tag="sW2_s")
                nc.gpsimd.dma_start(sW2_s, sW2.rearrange("e k m -> k e m"))
                sW3_s = cp.tile([HS2, E], BF16, name="sW3_s", tag="sW3_s")
                nc.gpsimd.dma_start(sW3_s, sW3.rearrange("e k o -> k (e o)"))
                tb3_s = cp.tile([E, OUT_T], BF16, name="tb3_s", tag="tb3_s")
                nc.gpsimd.dma_start(tb3_s, tb3)
                sb3_s = cp.tile([E, 1], BF16, name="sb3_s", tag="sb3_s")
                nc.gpsimd.dma_start(sb3_s, sb3)

                rb1_s = cp.tile([P, 2], F32, name="rb1_s", tag="rb1_s")
                nc.sync.dma_start(rb1_s, rb1.rearrange("(c p) -> p c", p=P))
                rb2_s = cp.tile([P, 1], F32, name="rb2_s", tag="rb2_s")
                nc.sync.dma_start(rb2_s, rb2[:, None])
                rb3_s = cp.tile([E, 1], F32, name="rb3_s", tag="rb3_s")
                nc.sync.dma_start(rb3_s, rb3[:, None])
                tb1_s = cp.tile([P, E, 2], F32, name="tb1_s", tag="tb1_s")
                nc.sync.dma_start(tb1_s, tb1.rearrange("e (c p) -> p e c", p=P))
                tb2_s = cp.tile([P, E, 2], F32, name="tb2_s", tag="tb2_s")
                nc.sync.dma_start(tb2_s, tb2.rearrange("e (c p) -> p e c", p=P))
                sb1_s = cp.tile([P, E], F32, name="sb1_s", tag="sb1_s")
                nc.sync.dma_start(sb1_s, sb1.rearrange("e p -> p e"))
                sb2_s = cp.tile([HS2, E], F32, name="sb2_s", tag="sb2_s")
                nc.sync.dma_start(sb2_s, sb2.rearrange("e p -> p e"))

            # ---- per 512-token tile ----
            for t in range(n_tiles):
                # load [512, 128] chunk token-major, PE-transpose to
                # feature-major xT [128 feat, 512 tok]
                xa = wp.tile([P, G, D], F32, name="xa", tag="xa")
                nc.sync.dma_start(
                    xa, x[ts(t, TILE), :].rearrange("(g p) d -> p g d", p=P))
                xT = wp.tile([P, TILE], F32, name="xT", tag="xT")
                for g in range(G):
                    xps = pp.tile([P, P], F32, name="xps", tag="rt", bufs=2)
                    nc.tensor.transpose(xps, xa[:, g, :], ident)
                    nc.vector.tensor_copy(xT[:, ts(g, P)], xps)
                xTb = wp.tile([P, TILE], BF16, name="xTb", tag="xTb")
                nc.vector.tensor_copy(xTb, xT)

                # ---- router MLP (fp32) ----
                r1a = pp.tile([P, TILE], F32, name="r1a", tag="rt", bufs=2)
                r1b = pp.tile([P, TILE], F32, name="r1b", tag="rt", bufs=2)
                mm(nc, r1a, rW1_s[:, 0:P], xT)
                mm(nc, r1b, rW1_s[:, P:R1], xT)
                h1 = wp.tile([P, 2, TILE], F32, name="h1", tag="h1")
                nc.scalar.activation(h1[:, 0, :], r1a, AF.Gelu, bias=rb1_s[:, 0:1])
                nc.scalar.activation(h1[:, 1, :], r1b, AF.Gelu, bias=rb1_s[:, 1:2])
                r2p = pp.tile([P, TILE], F32, name="r2p", tag="rt", bufs=2)
                mm(nc, r2p, rW2_s[:, 0, :], h1[:, 0, :], start=True, stop=False)
                mm(nc, r2p, rW2_s[:, 1, :], h1[:, 1, :], start=False, stop=True)
                h2 = wp.tile([P, TILE], F32, name="h2", tag="h2")
                nc.scalar.activation(h2, r2p, AF.Gelu, bias=rb2_s[:, 0:1])
                r3p = pp.tile([E, TILE], F32, name="r3p", tag="rt", bufs=2)
                mm(nc, r3p, rW3_s, h2)
                logit_fm = wp.tile([E, TILE], F32, name="logit_fm", tag="logit_fm")
                nc.vector.tensor_scalar(logit_fm, r3p, rb3_s, None, op0=ALU.add)

                # ---- logits to token-major, gating math ----
                ltp = pp.tile([P, G, E], F32, name="ltp", tag="rt", bufs=2)
                for g in range(G):
                    nc.tensor.transpose(ltp[:, g, :], logit_fm[:, ts(g, P)],
                                        ident[:E, :E])
                lt = wp.tile([P, G, E], F32, name="lt", tag="lt")
                nc.vector.tensor_copy(lt, ltp)

                m1 = wp.tile([P, G], F32, name="m1", tag="m1")
                nc.vector.reduce_max(m1, lt, axis=AX.X)
                m1b = m1[:, :, None].to_broadcast((P, G, E))
                top1 = wp.tile([P, G, E], F32, name="top1", tag="top1")
                nc.vector.tensor_tensor(top1, lt, m1b, op=ALU.is_ge)
                masked = wp.tile([P, G, E], F32, name="masked", tag="masked")
                nc.vector.tensor_scalar(masked, top1, -1e30, None, op0=ALU.mult)
                nc.vector.tensor_add(masked, masked, lt)
                m2 = wp.tile([P, G], F32, name="m2", tag="m2")
                nc.vector.reduce_max(m2, masked, axis=AX.X)
                m2b = m2[:, :, None].to_broadcast((P, G, E))

                # e1 = exp(lt - m1) via tanh (stays in the gelu table set):
                # s = sigmoid(z) = 0.5 + 0.5*tanh(z/2);  exp(z) = s / (1 - s)
                dd = wp.tile([P, G, E], F32, name="dd", tag="dd")
                nc.vector.tensor_sub(dd, lt, m1b)
                th = wp.tile([P, G, E], F32, name="th", tag="th")
                nc.scalar.activation(th, dd, AF.Tanh, scale=0.5)
                sg = wp.tile([P, G, E], F32, name="sg", tag="sg")
                nc.vector.tensor_scalar(sg, th, 0.5, 0.5, op0=ALU.mult, op1=ALU.add)
                omsg = wp.tile([P, G, E], F32, name="omsg", tag="omsg")
                nc.vector.tensor_scalar(omsg, th, -0.5, 0.5, op0=ALU.mult,
                                        op1=ALU.add)
                rn = wp.tile([P, G, E], F32, name="rn", tag="rn")
                nc.vector.reciprocal(rn, omsg)
                e1 = wp.tile([P, G, E], F32, name="e1", tag="e1")
                nc.vector.tensor_mul(e1, sg, rn)

                S = wp.tile([P, G], F32, name="S", tag="S")
                nc.vector.reduce_sum(S, e1, axis=AX.X)
                rS = wp.tile([P, G], F32, name="rS", tag="rS")
                nc.vector.reciprocal(rS, S)
                probs_t = wp.tile([P, G, E], F32, name="probs_t", tag="probs_t")
                nc.vector.tensor_mul(probs_t, e1,
                                     rS[:, :, None].to_broadcast((P, G, E)))
                with nc.allow_non_contiguous_dma(reason="24B prob rows"):
                    nc.sync.dma_start(
                        out_probs[ts(t, TILE), :].rearrange("(g p) e -> p g e", p=P),
                        probs_t)

                # top-2 renormalized weights:
                #   w[e] = exp(l[e]-m1) * (l[e] >= m2) / (1 + exp(m2-m1))
                # and 1/(1+exp(dm)) = sigmoid(-dm) = 0.5 - 0.5*tanh(dm/2)
                ge2 = wp.tile([P, G, E], F32, name="ge2", tag="ge2")
                nc.vector.tensor_tensor(ge2, lt, m2b, op=ALU.is_ge)
                dm = wp.tile([P, G], F32, name="dm", tag="dm")
                nc.vector.tensor_sub(dm, m2, m1)
                th2 = wp.tile([P, G], F32, name="th2", tag="th2")
                nc.scalar.activation(th2, dm, AF.Tanh, scale=0.5)
                rd = wp.tile([P, G], F32, name="rd", tag="rd")
                nc.vector.tensor_scalar(rd, th2, -0.5, 0.5, op0=ALU.mult,
                                        op1=ALU.add)
                wt = wp.tile([P, G, E], F32, name="wt", tag="wt")
                nc.vector.tensor_mul(wt, e1, ge2)
                nc.vector.tensor_mul(wt, wt, rd[:, :, None].to_broadcast((P, G, E)))

                # back to feature-major [E, 512], then broadcast across
                # partitions via stride-0 DRAM-source DMA
                wfp = pp.tile([E, TILE], F32, name="wfp", tag="rt", bufs=2)
                for g in range(G):
                    nc.tensor.transpose(wfp[:, ts(g, P)], wt[:, g, :], ident)
                w_fm = wp.tile([E, TILE], BF16, name="w_fm", tag="w_fm")
                nc.vector.tensor_copy(w_fm, wfp)
                w_dram = dp.tile([E, TILE], BF16, name="w_dram", tag="w_dram")
                nc.sync.dma_start(w_dram, w_fm)
                wb = wp.tile([P, E, TILE], BF16, name="wb", tag="wb", bufs=3)
                for e in range(E):
                    nc.sync.dma_start(
                        wb[:, e, :], w_dram[e:e + 1, :].to_broadcast((P, TILE)))

                # ---- expert heads (bf16), combine via PSUM accumulation ----
                traj_ps = pp.tile([OUT_T, TILE], F32, name="traj_ps",
                                  tag="acct", bufs=1)
                score_ps = pp.tile([1, TILE], F32, name="score_ps",
                                   tag="accs", bufs=1)
                for e in range(E):
                    # trajectory head
                    t1a = pp.tile([P, TILE], F32, name="t1a", tag="ps")
                    t1b = pp.tile([P, TILE], F32, name="t1b", tag="ps")
                    mm(nc, t1a, tW1_s[:, e, 0:P], xTb)
                    mm(nc, t1b, tW1_s[:, e, P:HT], xTb)
                    ht1 = wp.tile([P, 2, TILE], BF16, name="ht1", tag="ht1")
                    nc.scalar.activation(ht1[:, 0, :], t1a, AF.Gelu,
                                         bias=tb1_s[:, e, 0:1])
                    nc.scalar.activation(ht1[:, 1, :], t1b, AF.Gelu,
                                         bias=tb1_s[:, e, 1:2])
                    t2a = pp.tile([P, TILE], F32, name="t2a", tag="ps")
                    t2b = pp.tile([P, TILE], F32, name="t2b", tag="ps")
                    for c in range(2):
                        mm(nc, t2a, tW2_s[:, e, c, 0:P], ht1[:, c, :],
                           start=(c == 0), stop=(c == 1))
                        mm(nc, t2b, tW2_s[:, e, c, P:HT], ht1[:, c, :],
                           start=(c == 0), stop=(c == 1))
                    ht2w = wp.tile([P, 2, TILE], BF16, name="ht2w", tag="ht2w")
                    nc.scalar.activation(ht2w[:, 0, :], t2a, AF.Gelu,
                                         bias=tb2_s[:, e, 0:1])
                    nc.scalar.activation(ht2w[:, 1, :], t2b, AF.Gelu,
                                         bias=tb2_s[:, e, 1:2])
                    nc.vector.tensor_mul(ht2w[:, 0, :], ht2w[:, 0, :], wb[:, e, :])
                    nc.vector.tensor_mul(ht2w[:, 1, :], ht2w[:, 1, :], wb[:, e, :])
                    mm(nc, traj_ps, tW3_s[:, e, 0, :], ht2w[:, 0, :],
                       start=(e == 0), stop=False)
                    mm(nc, traj_ps, tW3_s[:, e, 1, :], ht2w[:, 1, :],
                       start=False, stop=False)

                    # score head
                    s1p = pp.tile([P, TILE], F32, name="s1p", tag="ps")
                    mm(nc, s1p, sW1_s[:, e, :], xTb)
                    hs1 = wp.tile([P, TILE], BF16, name="hs1", tag="hs1")
                    nc.scalar.activation(hs1, s1p, AF.Gelu, bias=sb1_s[:, e:e + 1])
                    s2p = pp.tile([HS2, TILE], F32, name="s2p", tag="ps")
                    mm(nc, s2p, sW2_s[:, e, :], hs1)
                    hs2w = wp.tile([HS2, TILE], BF16, name="hs2w", tag="hs2w")
                    nc.scalar.activation(hs2w, s2p, AF.Gelu, bias=sb2_s[:, e:e + 1])
                    nc.vector.tensor_mul(hs2w, hs2w, wb[0:HS2, e, :])
                    mm(nc, score_ps, sW3_s[:, e:e + 1], hs2w,
                       start=(e == 0), stop=False)

                # bias contributions: traj += tb3.T @ w_fm, score += sb3.T @ w_fm
                mm(nc, traj_ps, tb3_s, w_fm, start=False, stop=True)
                mm(nc, score_ps, sb3_s, w_fm, start=False, stop=True)

                # ---- outputs ----
                scoresb = wp.tile([1, TILE], F32, name="scoresb", tag="scoresb")
                nc.vector.tensor_copy(scoresb, score_ps)
                nc.sync.dma_start(out_score[ts(t, TILE)][None, :], scoresb)

                trajsb = wp.tile([OUT_T, TILE], F32, name="trajsb", tag="trajsb")
                nc.vector.tensor_copy(trajsb, traj_ps)
                outp = pp.tile([P, G, OUT_T], F32, name="outp", tag="ps")
                for g in range(G):
                    nc.tensor.transpose(outp[:, g, :], trajsb[:, ts(g, P)],
                                        ident[:OUT_T, :OUT_T])
                outsb = wp.tile([P, G, OUT_T], F32, name="outsb", tag="outsb")
                nc.vector.tensor_copy(outsb, outp)
                nc.sync.dma_start(
                    out_traj[ts(t, TILE), :].rearrange("(g p) o -> p g o", p=P),
                    outsb)
    nc.compile()
    return nc


_nc_cache: dict[int, bacc.Bacc] = {}


def get_nc(ntok: int) -> bacc.Bacc:
    if ntok not in _nc_cache:
        _nc_cache[ntok] = build_kernel(ntok)
    return _nc_cache[ntok]


last_results = None  # BassKernelResults of the most recent kernel() call


def _maybe_install_ntff_hook() -> bool:
    """Synthesize antenv.axon_hooks (absent on this image) so NTFF profiling
    works under axon.  Returns True when tracing is usable."""
    import sys
    import types

    try:
        from antenv.axon_hooks import get_axon_ntff_profile_hook  # noqa: F401
        return True
    except ImportError:
        pass
    try:
        import trn_agent_boot.trn_boot as tb

        hook = tb._ntff_profile_via_ctypes("/opt/axon/libaxon_pjrt.so")
        if hook is None:
            return False
        mod = types.ModuleType("antenv.axon_hooks")
        mod._hook = hook
        mod.get_axon_ntff_profile_hook = lambda: mod._hook
        mod.set_axon_ntff_profile_hook = lambda h: setattr(mod, "_hook", h)
        sys.modules["antenv.axon_hooks"] = mod
        import antenv

        antenv.axon_hooks = mod
        return True
    except Exception:
        return False


def kernel(**inputs) -> tuple:
    global last_results
    arrs = {k: np.ascontiguousarray(np.asarray(v, dtype=np.float32))
            for k, v in inputs.items()}
    x_full = arrs.pop("mode_features").reshape(NTOK, D)

    nc = get_nc(NTOK_CORE)
    in_maps = []
    for c in range(N_CORES):
        im = dict(arrs)
        im["x"] = x_full[c * NTOK_CORE:(c + 1) * NTOK_CORE]
        in_maps.append(im)

    want_trace = os.environ.get("BASS_KERNEL_TRACE", "0") == "1"
    trace = want_trace and _maybe_install_ntff_hook()
    if not trace:
        # keep a stray BASS_TRACE=1 in the environment from crashing the
        # axon trace path (it needs the hook module synthesized above)
        os.environ["BASS_NEVER_TRACE"] = "1"

    res = run_bass_kernel_spmd(
        nc, in_maps, core_ids=list(range(N_CORES)),
        trace=trace,
    )
    last_results = res

    traj = np.concatenate([r["out_traj"] for r in res.results], axis=0)
    score = np.concatenate([r["out_score"] for r in res.results], axis=0)
    probs = np.concatenate([r["out_probs"] for r in res.results], axis=0)

    trajectories = traj.reshape(B, M, T, 2)
    scores = score.reshape(B, M)
    probs_out = probs.reshape(B, M, E)
    avg = probs.reshape(NTOK, E).mean(axis=0)
    aux_loss = np.float32(E * np.sum(avg * avg))
    return trajectories, scores, aux_loss, probs_out
